# revision 10
# baseline (speedup 1.0000x reference)
"""Multi-head self-attention Bass/Tile kernel for TRN2.

Per-core problem (batch sharded across 8 cores):
  x [N=1024, C=768], Wqkv [768, 2304], bqkv [2304], Wproj [768, 768], bproj [768]
  -> y [1024, 768]

Design (all matmuls bf16, f32 PSUM accumulation; attention restructured so the
ACT engine streams 96 exp tiles back-to-back while PE interleaves everything):

  x   --DMA-->  f32 tiles --DVE cast--> bf16 --DMA-xbar-transpose--> xt [128,6,1024]
  qkT [128,1024] bf16 per feature M-tile (12), produced in query-half chunks
  V   [token,128-col] bf16 per token tile, stored padded per head with a
      64-wide ones block (PV then yields softmax sums replicated on rows 64:128)
  S^T [key128, (2j) x 512q] per (head, query-half, j-pair) in PSUM
  P   = exp(S*0.125 - 2) bf16 (ACT; the -2 shift cancels in normalization and
      is there so an fp8 variant can't overflow; harmless here)
  O   [128, 512] PSUM accumulated over 8 key tiles; rows 64:128 = sums
  aot = O[0:64] * recip(O[64:128]) -> bf16 (DVE, per head)
  y   = aot.T @ Wproj + (bproj + bqkv_v @ Wproj)  (V-bias folded into proj bias)

Scheduling: the attention loop (2 query-halves x 12 heads x 4 j-pairs) is the
spine; qkT/V/proj production matmuls are drained from a filler deque between
S/PV groups so the PE stream stays dense while ACT exps run concurrently.
"""

import numpy as np
from collections import deque

N = 1024
C = 768
H = 12
D = 64
NT = 8    # token tiles
CT = 6    # channel tiles
SCALE = 1.0 / np.sqrt(D)
ESHIFT = -2.0


def build_nc():
    import concourse.bass as bass
    import concourse.tile as tile
    from concourse import bacc, mybir

    f32 = mybir.dt.float32
    bf16 = mybir.dt.bfloat16
    Exp = mybir.ActivationFunctionType.Exp
    mul_op = mybir.AluOpType.mult
    add_op = mybir.AluOpType.add

    nc = bacc.Bacc(None, target_bir_lowering=False)

    x = nc.dram_tensor("x", [N, C], f32, kind="ExternalInput")
    wqkv = nc.dram_tensor("Wqkv", [C, 3 * C], f32, kind="ExternalInput")
    bqkv = nc.dram_tensor("bqkv", [3 * C], f32, kind="ExternalInput")
    wproj = nc.dram_tensor("Wproj", [C, C], f32, kind="ExternalInput")
    bproj = nc.dram_tensor("bproj", [C], f32, kind="ExternalInput")
    y = nc.dram_tensor("y", [N, C], f32, kind="ExternalOutput")

    mm = nc.tensor.matmul

    with tile.TileContext(nc) as tc:
        with (
            tc.tile_pool(name="const", bufs=1) as const,
            tc.tile_pool(name="xst", bufs=2) as xst_pool,
            tc.tile_pool(name="xbf", bufs=2) as xbf_pool,
            tc.tile_pool(name="xt", bufs=1) as xt_pool,
            tc.tile_pool(name="wst", bufs=2) as wst_pool,
            tc.tile_pool(name="wq", bufs=6) as wq_pool,
            tc.tile_pool(name="wp", bufs=2) as wp_pool,
            tc.tile_pool(name="qk", bufs=24) as qk_pool,
            tc.tile_pool(name="vpad", bufs=1) as vpad_pool,
            tc.tile_pool(name="pp", bufs=8) as p_pool,
            tc.tile_pool(name="aot", bufs=1) as aot_pool,
            tc.tile_pool(name="inv", bufs=3) as inv_pool,
            tc.tile_pool(name="yb", bufs=2) as y_pool,
            tc.tile_pool(name="psS", bufs=2, space="PSUM") as psS,
            tc.tile_pool(name="psO", bufs=2, space="PSUM") as psO,
            tc.tile_pool(name="psP", bufs=2, space="PSUM") as psP,
        ):
            # ---------------- constants (small, Pool queue) ----------------
            bq_cols = const.tile([128, 12], f32)
            nc.gpsimd.dma_start(
                bq_cols[:], bqkv.ap().rearrange("(m p) -> p m", p=128)[:, 0:12]
            )
            bv_col = const.tile([128, CT], f32)
            nc.gpsimd.dma_start(
                bv_col[:], bqkv.ap()[2 * C : 3 * C].rearrange("(k p) -> p k", p=128)
            )
            bv_bf = const.tile([128, CT], bf16)
            nc.vector.tensor_copy(bv_bf[:], bv_col[:])
            bp_row = const.tile([1, C], f32)
            nc.gpsimd.dma_start(bp_row[:], bproj.ap()[None, :])
            brow_sb = const.tile([1, C], f32)
            brow_bc = const.tile([128, C], f32)
            eshift_col = const.tile([128, 1], f32)
            nc.vector.memset(eshift_col[:], ESHIFT)

            # vpad: [128 tok, (j 8) (h 12) (m 128)] bf16; cols 64:128 = ones
            vpad = vpad_pool.tile([128, NT * H * 128], bf16)
            vp = vpad[:].rearrange("p (j h m) -> p j h m", j=NT, h=H)
            nc.gpsimd.memset(vp[:, :, :, 64:128], 1.0)

            # ---------------- x pipeline: load, cast, xbar-transpose -------
            xt_all = xt_pool.tile([128, CT * N], bf16)
            xt_v = xt_all[:].rearrange("p (k n) -> p k n", k=CT)
            pending_T = []

            def x_tile(ti):
                xst = xst_pool.tile([128, C], f32, name=f"xst{ti}", tag="xst")
                nc.sync.dma_start(xst[:], x.ap()[ti * 128 : (ti + 1) * 128, :])
                xbf = xbf_pool.tile([128, C], bf16, name=f"xbf{ti}", tag="xbf")
                nc.vector.tensor_copy(xbf[:], xst[:])
                pending_T.append(
                    lambda ti=ti, xbf=xbf: nc.sync.dma_start_transpose(
                        xt_v[:, :, ti * 128 : (ti + 1) * 128], xbf[:]
                    )
                )

            # loads run ahead of transposes on the sync queue so a transpose's
            # wait (on the DVE cast) never delays a later x load
            for ti in range(4):
                x_tile(ti)
            for ti in range(4, NT):
                pending_T.pop(0)()
                x_tile(ti)
            while pending_T:
                pending_T.pop(0)()

            # ---------------- weight loads + casts -------------------------
            # Wqkv columns, bf16, split per column-group tile:
            #   groups 0..5: q|k M-tile pairs (256 cols each)
            #   groups 6,7:  v columns (384 each)
            wqg = {}

            def w_group(gi, c0, w, eng):
                cw = 384 if c0 >= 2 * C else 256
                st = wst_pool.tile([128, 2304], f32, name=f"wst{gi}", tag="wst")
                stv = st[:, 0 : CT * cw].rearrange("p (k c) -> p k c", k=CT)
                nc.scalar.dma_start(
                    stv,
                    wqkv.ap().rearrange("(k p) c -> p k c", p=128)[:, :, c0 : c0 + cw],
                )
                wt = w.tile(
                    [128, CT * cw], bf16, name=f"wq{gi}", tag=f"wq{cw}",
                    bufs=(2 if cw == 384 else 6),
                )
                eng.tensor_copy(wt[:], st[:, 0 : CT * cw])
                wqg[gi] = wt[:].rearrange("p (k c) -> p k c", k=CT)

            # q|k groups for M-tile pair (t, t+1) live in wqg[t//2] etc.
            # load order: by attention demand.
            w_group(3, 768, wq_pool, nc.vector)    # Mtiles 6,7 (k, heads 0-3)
            w_group(0, 0, wq_pool, nc.vector)      # Mtiles 0,1 (q, heads 0-3)
            w_group(6, 1536, wq_pool, nc.gpsimd)   # v cols 0:384
            w_group(7, 1920, wq_pool, nc.gpsimd)   # v cols 384:768
            w_group(4, 1024, wq_pool, nc.gpsimd)   # Mtiles 8,9
            w_group(1, 256, wq_pool, nc.gpsimd)    # Mtiles 2,3
            w_group(5, 1280, wq_pool, nc.gpsimd)   # Mtiles 10,11
            w_group(2, 512, wq_pool, nc.gpsimd)    # Mtiles 4,5

            def qk_lhsT(mi, k):  # [128, 128] W tile for feature M-tile mi
                g = mi // 2 if mi < 6 else 3 + (mi - 6) // 2
                return wqg[g][:, k, (mi % 2) * 128 : (mi % 2) * 128 + 128]

            # Wproj halves (deferred; loaded during attention)
            wpg = {}

            def wp_group(hi):
                st = wst_pool.tile([128, 2304], f32, name=f"wpst{hi}", tag="wst")
                stv = st[:, 0 : CT * 384].rearrange("p (k c) -> p k c", k=CT)
                nc.sync.dma_start(
                    stv,
                    wproj.ap().rearrange("(k p) c -> p k c", p=128)[
                        :, :, hi * 384 : (hi + 1) * 384
                    ],
                )
                wt = wp_pool.tile([128, CT * 384], bf16, name=f"wp{hi}", tag="wp")
                nc.gpsimd.tensor_copy(wt[:], st[:, 0 : CT * 384])
                wpg[hi] = wt[:].rearrange("p (k c) -> p k c", k=CT)

            # ---------------- persistent bf16 activation tiles -------------
            # qkT half-tiles: qkTh[mi][half] = [128, 512] (tokens half*512..)
            qkTh = [
                [
                    qk_pool.tile([128, 512], bf16, name=f"qkT{mi}_{hf}", tag="qkT")
                    for hf in range(2)
                ]
                for mi in range(12)
            ]
            aot = [aot_pool.tile([128, CT * 512], bf16, name=f"aot{qh}") for qh in range(2)]
            aot_v = [a[:].rearrange("p (k n) -> p k n", k=CT) for a in aot]

            # ---------------- production units (filler closures) -----------
            def u_qk(mi, hf):
                def emit():
                    ps = psP.tile([128, 512], f32, name=f"qps{mi}_{hf}", tag="ps")
                    for k in range(CT):
                        mm(ps[:], qk_lhsT(mi, k), xt_v[:, k, hf * 512 : hf * 512 + 512],
                           start=(k == 0), stop=(k == CT - 1))
                    nc.vector.tensor_scalar_add(
                        qkTh[mi][hf][:], ps[:], bq_cols[:, mi : mi + 1]
                    )
                return emit

            def u_v(ti, hf):  # V chunk: channels hf*384..+384 = heads hf*6..+6
                def emit():
                    ps = psP.tile([128, 512], f32, name=f"vps{ti}_{hf}", tag="ps")
                    for k in range(CT):
                        mm(ps[:, 0:384], xt_v[:, k, ti * 128 : (ti + 1) * 128],
                           wqg[6 + hf][:, k, :],
                           start=(k == 0), stop=(k == CT - 1))
                    nc.vector.tensor_copy(
                        vp[:, ti, hf * 6 : hf * 6 + 6, 0:64],
                        ps[:, 0:384].rearrange("p (h d) -> p h d", h=6),
                    )
                return emit

            def u_wp(hi):
                return lambda: wp_group(hi)

            def u_brow(hi):
                def emit():
                    ps = psP.tile([128, 512], f32, name=f"brps{hi}", tag="ps")
                    for k in range(CT):
                        mm(ps[0:1, 0:384], bv_bf[:, k : k + 1], wpg[hi][:, k, :],
                           start=(k == 0), stop=(k == CT - 1))
                    nc.vector.tensor_tensor(
                        out=brow_sb[:, hi * 384 : (hi + 1) * 384],
                        in0=ps[0:1, 0:384],
                        in1=bp_row[:, hi * 384 : (hi + 1) * 384],
                        op=add_op,
                    )
                    if hi == 1:
                        nc.gpsimd.partition_broadcast(brow_bc[:], brow_sb[:])
                return emit

            def u_proj(t, hi):  # token tile t, output cols hi*384..+384
                def emit():
                    qh, tt = t // 4, t % 4
                    ps = psP.tile([128, 512], f32, name=f"yps{t}_{hi}", tag="ps")
                    for k in range(CT):
                        mm(ps[:, 0:384], aot_v[qh][:, k, tt * 128 : (tt + 1) * 128],
                           wpg[hi][:, k, :],
                           start=(k == 0), stop=(k == CT - 1))
                    yt = ytiles[t]
                    nc.vector.tensor_tensor(
                        out=yt[:, hi * 384 : (hi + 1) * 384],
                        in0=ps[:, 0:384],
                        in1=brow_bc[:, hi * 384 : (hi + 1) * 384],
                        op=add_op,
                    )
                    if hi == 1:
                        (nc.sync if t % 2 == 0 else nc.gpsimd).dma_start(
                            y.ap()[t * 128 : (t + 1) * 128, :], yt[:]
                        )
                return emit

            ytiles = {}
            for t in range(NT):
                ytiles[t] = y_pool.tile([128, C], f32, name=f"yt{t}", tag="yt")

            # ---------------- prologue production ---------------------------
            # First heads need qkT Mtiles 6 (both halves: keys) + 0 (query half 0),
            # and PV(h0) needs V tiles rolling in.
            u_qk(6, 0)(); u_qk(6, 1)(); u_qk(0, 0)()
            u_v(0, 0)(); u_v(0, 1)(); u_v(1, 0)(); u_v(1, 1)()

            # ---------------- filler schedules ------------------------------
            # NOTE: fillers are popped in emission order and the Tile framework
            # only enforces dependencies backwards in program order, so each
            # V half-tile unit must be EMITTED before the first PV matmul that
            # reads it (PV of head h at j-pair jp reads tiles 2jp,2jp+1 of the
            # heads hf*6..hf*6+6 half it belongs to).
            fill_qh0 = deque()
            for unit in (
                u_v(2, 0), u_v(3, 0), u_v(4, 0), u_v(5, 0), u_v(6, 0), u_v(7, 0),
                u_qk(7, 0), u_qk(7, 1), u_qk(1, 0),
                u_v(2, 1), u_v(3, 1), u_v(4, 1), u_v(5, 1), u_v(6, 1), u_v(7, 1),
                u_qk(8, 0), u_qk(8, 1), u_qk(2, 0),
                u_qk(9, 0), u_qk(9, 1), u_qk(3, 0),
                u_qk(10, 0), u_qk(10, 1), u_qk(4, 0),
                u_qk(11, 0), u_qk(11, 1), u_qk(5, 0),
                u_qk(0, 1), u_qk(1, 1),
            ):
                fill_qh0.append(unit)
            fill_qh1 = deque()
            for unit in (
                u_qk(2, 1), u_qk(3, 1),
                u_wp(0), u_wp(1),
                u_qk(4, 1), u_qk(5, 1),
                u_brow(0), u_brow(1),
                u_proj(0, 0), u_proj(0, 1), u_proj(1, 0), u_proj(1, 1),
                u_proj(2, 0), u_proj(2, 1), u_proj(3, 0), u_proj(3, 1),
            ):
                fill_qh1.append(unit)

            # ---------------- attention spine -------------------------------
            for qh in range(2):
                fillers = fill_qh0 if qh == 0 else fill_qh1
                for h in range(H):
                    t, h2 = h // 2, h % 2
                    hsl = slice(h2 * 64, h2 * 64 + 64)
                    O = psO.tile([128, 512], f32, name=f"O{qh}_{h}", tag="O")
                    for jp in range(4):
                        S = psS.tile([128, 1024], f32, name=f"S{qh}_{h}_{jp}", tag="S")
                        for jj in range(2):
                            j = 2 * jp + jj
                            mm(S[:, jj * 512 : jj * 512 + 512],
                               qkTh[6 + t][j // 4][hsl, (j % 4) * 128 : (j % 4) * 128 + 128],
                               qkTh[t][qh][hsl, :],
                               start=True, stop=True)
                        if fillers:
                            fillers.popleft()()
                        P = p_pool.tile([128, 1024], bf16, name=f"P{qh}_{h}_{jp}", tag="P")
                        nc.scalar.activation(
                            P[:], S[:], Exp, bias=eshift_col[:, 0:1], scale=SCALE
                        )
                        for jj in range(2):
                            j = 2 * jp + jj
                            mm(O[:], vp[:, j, h, :], P[:, jj * 512 : jj * 512 + 512],
                               start=(jp == 0 and jj == 0),
                               stop=(jp == 3 and jj == 1))
                        if fillers:
                            fillers.popleft()()
                    inv = inv_pool.tile([64, 512], f32, name=f"inv{qh}_{h}", tag="inv")
                    nc.vector.reciprocal(inv[:], O[64:128, :])
                    nc.vector.tensor_tensor(
                        out=aot_v[qh][h2 * 64 : h2 * 64 + 64, t, :],
                        in0=O[0:64, :],
                        in1=inv[:],
                        op=mul_op,
                    )
                while fillers:
                    fillers.popleft()()

            # ---------------- tail: proj for query half 1 -------------------
            for t in range(4, NT):
                u_proj(t, 0)()
                u_proj(t, 1)()

    nc.compile()
    return nc


_NC_CACHE = {}


def _get_nc():
    nc = _NC_CACHE.get("nc")
    if nc is None:
        nc = build_nc()
        _NC_CACHE["nc"] = nc
    return nc


_RUNNER_CACHE = {}
_DEV_CACHE = {}


def _get_runner(n_cores=8):
    """Cached jitted 8-core executor (PJRT path, no per-call retrace)."""
    if n_cores in _RUNNER_CACHE:
        return _RUNNER_CACHE[n_cores]
    import jax
    from jax.sharding import Mesh, PartitionSpec
    from jax.experimental.shard_map import shard_map
    from concourse import mybir
    from concourse.bass2jax import (
        _bass_exec_p,
        install_neuronx_cc_hook,
        partition_id_tensor,
    )

    nc = _get_nc()
    install_neuronx_cc_hook()
    partition_name = nc.partition_id_tensor.name if nc.partition_id_tensor else None

    in_names, out_names, out_avals = [], [], []
    for alloc in nc.m.functions[0].allocations:
        if not isinstance(alloc, mybir.MemoryLocationSet):
            continue
        name = alloc.memorylocations[0].name
        if alloc.kind == "ExternalInput":
            if name != partition_name:
                in_names.append(name)
        elif alloc.kind == "ExternalOutput":
            out_names.append(name)
            out_avals.append(
                jax.core.ShapedArray(
                    tuple(alloc.tensor_shape), mybir.dt.np(alloc.dtype)
                )
            )
    all_in_names = list(in_names)
    if partition_name is not None:
        all_in_names.append(partition_name)

    def _body(*args):
        operands = list(args)
        if partition_name is not None:
            operands.append(partition_id_tensor())
        return tuple(
            _bass_exec_p.bind(
                *operands,
                out_avals=tuple(out_avals),
                in_names=tuple(all_in_names),
                out_names=tuple(out_names),
                lowering_input_output_aliases=(),
                sim_require_finite=False,
                sim_require_nnan=False,
                nc=nc,
            )
        )

    devices = jax.devices()[:n_cores]
    mesh = Mesh(np.asarray(devices), ("core",))
    in_specs = tuple(
        PartitionSpec("core") if n == "x" else PartitionSpec() for n in in_names
    )
    fn = jax.jit(
        shard_map(
            _body,
            mesh=mesh,
            in_specs=in_specs,
            out_specs=(PartitionSpec("core"),) * len(out_names),
            check_rep=False,
        ),
        keep_unused=True,
    )
    _RUNNER_CACHE[n_cores] = (fn, in_names, mesh)
    return _RUNNER_CACHE[n_cores]


def kernel(x, Wqkv, bqkv, Wproj, bproj):
    """Full-input entry point.

    x [8, 1024, 768] is sharded one batch element per NeuronCore (data
    parallel, weights replicated, no collectives); outputs are re-stacked.
    """
    x = np.ascontiguousarray(np.asarray(x, dtype=np.float32))
    Wqkv = np.ascontiguousarray(np.asarray(Wqkv, dtype=np.float32))
    bqkv = np.ascontiguousarray(np.asarray(bqkv, dtype=np.float32))
    Wproj = np.ascontiguousarray(np.asarray(Wproj, dtype=np.float32))
    bproj = np.ascontiguousarray(np.asarray(bproj, dtype=np.float32))
    B = x.shape[0]
    assert x.shape == (8, N, C), f"expected (8, {N}, {C}), got {x.shape}"

    arrays = {
        "x": x.reshape(B * N, C),
        "Wqkv": Wqkv,
        "bqkv": bqkv,
        "Wproj": Wproj,
        "bproj": bproj,
    }
    try:
        import jax
        from jax.sharding import NamedSharding, PartitionSpec

        fn, in_names, mesh = _get_runner(B)
        ops = []
        for n in in_names:
            a = arrays[n]
            if n == "x":
                ops.append(a)
                continue
            key = (n, id(a), a.shape)
            cached = _DEV_CACHE.get(n)
            if cached is None or cached[0] != key:
                dev = jax.device_put(a, NamedSharding(mesh, PartitionSpec()))
                _DEV_CACHE[n] = (key, dev, a)
                cached = _DEV_CACHE[n]
            ops.append(cached[1])
        outs = fn(*ops)
        yv = np.asarray(outs[0]).reshape(B, N, C)
        return yv.astype(np.float32)
    except Exception:
        from concourse import bass_utils

        nc = _get_nc()
        in_maps = [
            {
                "x": x[c],
                "Wqkv": Wqkv,
                "bqkv": bqkv,
                "Wproj": Wproj,
                "bproj": bproj,
            }
            for c in range(B)
        ]
        res = bass_utils.run_bass_kernel_spmd(nc, in_maps, core_ids=list(range(B)))
        return np.stack([res.results[c]["y"] for c in range(B)]).astype(np.float32)


# revision 37
# speedup vs baseline: 1.1415x; 1.1415x over previous
"""Multi-head self-attention Bass/Tile kernel for TRN2.

Per-core problem (batch sharded across 8 cores):
  x [N=1024, C=768], Wqkv [768, 2304], bqkv [2304], Wproj [768, 768], bproj [768]
  -> y [1024, 768]

Design (all matmuls bf16, f32 PSUM accumulation; attention restructured so the
ACT engine streams 96 exp tiles back-to-back while PE interleaves everything):

  x   --DMA-->  f32 tiles --DVE cast--> bf16 --DMA-xbar-transpose--> xt [128,6,1024]
  qkT [128,1024] bf16 per feature M-tile (12), produced in query-half chunks
  V   [token,128-col] bf16 per token tile, stored padded per head with a
      64-wide ones block (PV then yields softmax sums replicated on rows 64:128)
  S^T [key128, (2j) x 512q] per (head, query-half, j-pair) in PSUM
  P   = exp(S*0.125 - 2) bf16 (ACT; the -2 shift cancels in normalization and
      is there so an fp8 variant can't overflow; harmless here)
  O   [128, 512] PSUM accumulated over 8 key tiles; rows 64:128 = sums
  aot = O[0:64] * recip(O[64:128]) -> bf16 (DVE, per head)
  y   = aot.T @ Wproj + (bproj + bqkv_v @ Wproj)  (V-bias folded into proj bias)

Scheduling: the attention loop (2 query-halves x 12 heads x 4 j-pairs) is the
spine; qkT/V/proj production matmuls are drained from a filler deque between
S/PV groups so the PE stream stays dense while ACT exps run concurrently.
"""

import numpy as np
from collections import deque

N = 1024
C = 768
H = 12
D = 64
NT = 8    # token tiles
CT = 6    # channel tiles
SCALE = 1.0 / np.sqrt(D)
ESHIFT = -2.0


def build_nc():
    import concourse.bass as bass
    import concourse.tile as tile
    from concourse import bacc, mybir

    f32 = mybir.dt.float32
    bf16 = mybir.dt.bfloat16
    fp8 = mybir.dt.float8e4
    DR = mybir.MatmulPerfMode.DoubleRow
    Exp = mybir.ActivationFunctionType.Exp
    mul_op = mybir.AluOpType.mult
    add_op = mybir.AluOpType.add
    sub_op = mybir.AluOpType.subtract

    nc = bacc.Bacc(None, target_bir_lowering=False)

    x = nc.dram_tensor("x", [N, C], f32, kind="ExternalInput")
    wqkv = nc.dram_tensor("Wqkv", [C, 3 * C], f32, kind="ExternalInput")
    bqkv = nc.dram_tensor("bqkv", [3 * C], f32, kind="ExternalInput")
    wproj = nc.dram_tensor("Wproj", [C, C], f32, kind="ExternalInput")
    bproj = nc.dram_tensor("bproj", [C], f32, kind="ExternalInput")
    y = nc.dram_tensor("y", [N, C], f32, kind="ExternalOutput")

    mm = nc.tensor.matmul

    from contextlib import ExitStack
    with ExitStack() as _es:
        tc = _es.enter_context(tile.TileContext(nc))
        P = lambda **kw: _es.enter_context(tc.tile_pool(**kw))
        const = P(name="const", bufs=1)
        xst_pool = P(name="xst", bufs=4)
        xbf_pool = P(name="xbf", bufs=6)
        xt_pool = P(name="xt", bufs=1)
        x8_pool = P(name="x8", bufs=1)
        w64_pool = P(name="w64", bufs=2)
        wst_pool = P(name="wst", bufs=2)
        wq_pool = P(name="wq", bufs=6)
        wp_pool = P(name="wp", bufs=2)
        qk_pool = P(name="qk", bufs=24)
        vpad_pool = P(name="vpad", bufs=1)
        p_pool = P(name="pp", bufs=8)
        aot_pool = P(name="aot", bufs=1)
        inv_pool = P(name="inv", bufs=2)
        y_pool = P(name="yb", bufs=2)
        psS = P(name="psS", bufs=2, space="PSUM")
        psO = P(name="psO", bufs=2, space="PSUM")
        psP = P(name="psP", bufs=2, space="PSUM")
        if True:
            bq_cols = const.tile([128, 12], f32)
            nc.gpsimd.dma_start(
                bq_cols[:], bqkv.ap().rearrange("(m p) -> p m", p=128)[:, 0:12]
            )
            bv_col = const.tile([128, CT], f32)
            nc.gpsimd.dma_start(
                bv_col[:], bqkv.ap()[2 * C : 3 * C].rearrange("(k p) -> p k", p=128)
            )
            bv_bf = const.tile([128, CT], bf16)
            nc.vector.tensor_copy(bv_bf[:], bv_col[:])
            bp_row = const.tile([1, C], f32)
            brow_sb = const.tile([1, C], f32)
            brow_bc = const.tile([128, C], f32)
            eshift_col = const.tile([128, 1], f32)
            nc.vector.memset(eshift_col[:], ESHIFT)

            # vpad: [128 tok, (j 8) (h 12) (m 128)] bf16; cols 64:128 = ones
            vpad = vpad_pool.tile([128, NT * H * 128], bf16)
            vp = vpad[:].rearrange("p (j h m) -> p j h m", j=NT, h=H)
            nc.gpsimd.memset(vp[:, :, :, 64:128], 1.0)

            # ---------------- x pipeline: load, cast, xbar-transpose -------
            xt_all = xt_pool.tile([128, CT * N], bf16)
            xt_v = xt_all[:].rearrange("p (k n) -> p k n", k=CT)
            x0t = x8_pool.tile([128, CT * N], fp8, name="x0t", tag="x0t")
            x1t = x8_pool.tile([128, CT * N], fp8, name="x1t", tag="x1t")
            x0t_v = x0t[:].rearrange("p (k n) -> p k n", k=CT)
            x1t_v = x1t[:].rearrange("p (k n) -> p k n", k=CT)
            pending_T = []

            def u_derive(ti):  # xt -> fp8 residual pair for token tile ti
                def emit():
                    sl = slice(ti * 128, (ti + 1) * 128)
                    nc.vector.tensor_copy(x0t_v[:, :, sl], xt_v[:, :, sl])
                    nc.vector.tensor_tensor(
                        out=x1t_v[:, :, sl], in0=xt_v[:, :, sl],
                        in1=x0t_v[:, :, sl], op=sub_op,
                    )
                return emit

            def x_tile(ti, eng=None):
                xst = xst_pool.tile([128, C], f32, name=f"xst{ti}", tag="xst")
                (eng or nc.sync).dma_start(xst[:], x.ap()[ti * 128 : (ti + 1) * 128, :])
                xbf = xbf_pool.tile([128, C], bf16, name=f"xbf{ti}", tag="xbf")
                nc.vector.tensor_copy(xbf[:], xst[:])
                pending_T.append(
                    lambda ti=ti, xbf=xbf: nc.sync.dma_start_transpose(
                        xt_v[:, :, ti * 128 : (ti + 1) * 128], xbf[:]
                    )
                )

            # loads run ahead of transposes on the sync queue so a transpose's
            # wait (on the DVE cast) never delays a later x load; token half 0
            # (tiles 0-3) transposes first since S needs key-half 0 first
            for ti in range(4):
                x_tile(ti)
            while pending_T:
                pending_T.pop(0)()

            # ---------------- weight loads + casts -------------------------
            # Wqkv columns, bf16, split per column-group tile:
            #   groups 0..5: q|k M-tile pairs (256 cols each)
            #   groups 6,7:  v columns (384 each)
            wqg = {}   # v-column groups (6: cols 0:384, 7: cols 384:768)
            wqm = {}   # bf16 per-Mtile q|k weight tiles (prologue Mtiles 0, 6)
            wqr = {}   # residual fp8 pairs (w0, w1) of 64*W per Mtile, viewed
                       # [128, 3 cpairs, 2, 128] for DoubleRow lhsT

            def w_mtile(mi, eng_dma, eng_cast):
                st = wst_pool.tile([128, 2304], f32, name=f"wsm{mi}", tag="wst")
                stv = st[:, 0 : CT * 128].rearrange("p (k c) -> p k c", k=CT)
                eng_dma.dma_start(
                    stv,
                    wqkv.ap().rearrange("(k p) c -> p k c", p=128)[
                        :, :, mi * 128 : (mi + 1) * 128
                    ],
                )
                wt = wq_pool.tile(
                    [128, CT * 128], bf16, name=f"wm{mi}", tag="wm", bufs=2
                )
                eng_cast.tensor_copy(wt[:], st[:, 0 : CT * 128])
                wqm[mi] = wt[:].rearrange("p (k c) -> p k c", k=CT)

            def w_mtile_res(mi, eng_dma, eng_chain):
                # 64*W as an fp8 residual pair (the x64 dodges e4m3 subnormals
                # on W ~ N(0, C^-0.5); the qkT eviction scales psum by 1/64)
                st = wst_pool.tile([128, 2304], f32, name=f"wsm{mi}", tag="wst")
                stv = st[:, 0 : CT * 128].rearrange("p (k c) -> p k c", k=CT)
                eng_dma.dma_start(
                    stv,
                    wqkv.ap().rearrange("(k p) c -> p k c", p=128)[
                        :, :, mi * 128 : (mi + 1) * 128
                    ],
                )
                st64 = w64_pool.tile([128, CT * 128], f32, name=f"w64_{mi}", tag="w64")
                eng_chain.tensor_scalar_mul(st64[:], st[:, 0 : CT * 128], 64.0)
                w0 = wq_pool.tile([128, CT * 128], fp8, name=f"w0_{mi}", tag="w0", bufs=12)
                eng_chain.tensor_copy(w0[:], st64[:])
                w1 = wq_pool.tile([128, CT * 128], fp8, name=f"w1_{mi}", tag="w1", bufs=12)
                eng_chain.tensor_tensor(out=w1[:], in0=st64[:], in1=w0[:], op=sub_op)
                wqr[mi] = tuple(
                    w[:].rearrange("p (u i c) -> p u i c", u=3, i=2) for w in (w0, w1)
                )

            def w_group(gi, c0, eng_dma, eng_cast):
                cw = 384
                st = wst_pool.tile([128, 2304], f32, name=f"wst{gi}", tag="wst")
                stv = st[:, 0 : CT * cw].rearrange("p (k c) -> p k c", k=CT)
                eng_dma.dma_start(
                    stv,
                    wqkv.ap().rearrange("(k p) c -> p k c", p=128)[:, :, c0 : c0 + cw],
                )
                wt = wq_pool.tile(
                    [128, CT * cw], bf16, name=f"wq{gi}", tag="wq384", bufs=2
                )
                eng_cast.tensor_copy(wt[:], st[:, 0 : CT * cw])
                wqg[gi] = wt[:].rearrange("p (k c) -> p k c", k=CT)

            # First-needed M-tiles on the scalar queue right away; the rest of
            # the x pipeline; then the heads-0-5 V columns on sync after T7 so
            # the transposes (S key-half 1) aren't delayed.
            w_mtile(6, nc.scalar, nc.vector)
            w_mtile(0, nc.scalar, nc.vector)
            for ti in range(4, NT):
                x_tile(ti)
            while pending_T:
                pending_T.pop(0)()
            nc.scalar.dma_start(bp_row[:], bproj.ap()[None, :])
            w_group(6, 1536, nc.sync, nc.vector)   # v cols 0:384 (heads 0-5)

            def qk_lhsT(mi, k):  # [128, 128] W tile for feature M-tile mi
                return wqm[mi][:, k, :]

            # Wproj halves (deferred; loaded during attention)
            wpg = {}

            def wp_group(hi):
                st = wst_pool.tile([128, 2304], f32, name=f"wpst{hi}", tag="wst")
                stv = st[:, 0 : CT * 384].rearrange("p (k c) -> p k c", k=CT)
                nc.sync.dma_start(
                    stv,
                    wproj.ap().rearrange("(k p) c -> p k c", p=128)[
                        :, :, hi * 384 : (hi + 1) * 384
                    ],
                )
                wt = wp_pool.tile([128, CT * 384], bf16, name=f"wp{hi}", tag="wp")
                nc.gpsimd.tensor_copy(wt[:], st[:, 0 : CT * 384])
                wpg[hi] = wt[:].rearrange("p (k c) -> p k c", k=CT)

            # ---------------- persistent bf16 activation tiles -------------
            # qkT half-tiles: qkTh[mi][half] = [128, 512] (tokens half*512..)
            qkTh = [
                [
                    qk_pool.tile([128, 512], bf16, name=f"qkT{mi}_{hf}", tag="qkT")
                    for hf in range(2)
                ]
                for mi in range(12)
            ]
            aot = [aot_pool.tile([128, CT * 512], bf16, name=f"aot{qh}") for qh in range(2)]
            aot_v = [a[:].rearrange("p (k n) -> p k n", k=CT) for a in aot]

            # ---------------- production units (filler closures) -----------
            def u_qk(mi, hf):
                def emit():
                    ps = psP.tile([128, 512], f32, name=f"qps{mi}_{hf}", tag="ps")
                    sl = slice(hf * 512, hf * 512 + 512)
                    if mi in wqm:  # bf16 path (prologue M-tiles)
                        for k in range(CT):
                            mm(ps[:], qk_lhsT(mi, k), xt_v[:, k, sl],
                               start=(k == 0), stop=(k == CT - 1))
                        nc.vector.tensor_scalar_add(
                            qkTh[mi][hf][:], ps[:], bq_cols[:, mi : mi + 1]
                        )
                        return
                    # fp8 residual DoubleRow path: 64*(q|k) = x0W0 + x1W0 + x0W1
                    w0, w1 = wqr[mi]
                    passes = ((w0, x0t_v), (w0, x1t_v), (w1, x0t_v))
                    n9 = 0
                    for wv, xv in passes:
                        for u in range(3):
                            mm(ps[:], wv[:, u, :, :],
                               xv[:, 2 * u : 2 * u + 2, sl],
                               start=(n9 == 0), stop=(n9 == 8), perf_mode=DR)
                            n9 += 1
                    nc.vector.tensor_scalar(
                        qkTh[mi][hf][:], ps[:], 1.0 / 64.0,
                        bq_cols[:, mi : mi + 1], mul_op, add_op,
                    )
                return emit

            def u_v(ti, hf):  # V chunk: channels hf*384..+384 = heads hf*6..+6
                def emit():
                    ps = psP.tile([128, 512], f32, name=f"vps{ti}_{hf}", tag="ps")
                    for k in range(CT):
                        mm(ps[:, 0:384], xt_v[:, k, ti * 128 : (ti + 1) * 128],
                           wqg[6 + hf][:, k, :],
                           start=(k == 0), stop=(k == CT - 1))
                    nc.vector.tensor_copy(
                        vp[:, ti, hf * 6 : hf * 6 + 6, 0:64],
                        ps[:, 0:384].rearrange("p (h d) -> p h d", h=6),
                    )
                return emit

            def u_wp(hi):
                return lambda: wp_group(hi)

            def u_brow(hi):
                def emit():
                    ps = psP.tile([128, 512], f32, name=f"brps{hi}", tag="ps")
                    for k in range(CT):
                        mm(ps[0:1, 0:384], bv_bf[:, k : k + 1], wpg[hi][:, k, :],
                           start=(k == 0), stop=(k == CT - 1))
                    nc.vector.tensor_tensor(
                        out=brow_sb[:, hi * 384 : (hi + 1) * 384],
                        in0=ps[0:1, 0:384],
                        in1=bp_row[:, hi * 384 : (hi + 1) * 384],
                        op=add_op,
                    )
                    if hi == 1:
                        nc.gpsimd.partition_broadcast(brow_bc[:], brow_sb[:])
                return emit

            def u_proj(t, hi):  # token tile t, output cols hi*384..+384
                def emit():
                    qh, tt = t // 4, t % 4
                    ps = psP.tile([128, 512], f32, name=f"yps{t}_{hi}", tag="ps")
                    for k in range(CT):
                        mm(ps[:, 0:384], aot_v[qh][:, k, tt * 128 : (tt + 1) * 128],
                           wpg[hi][:, k, :],
                           start=(k == 0), stop=(k == CT - 1))
                    yt = ytiles[t]
                    nc.vector.tensor_tensor(
                        out=yt[:, hi * 384 : (hi + 1) * 384],
                        in0=ps[:, 0:384],
                        in1=brow_bc[:, hi * 384 : (hi + 1) * 384],
                        op=add_op,
                    )
                    if hi == 1:
                        (nc.sync if t % 2 == 0 else nc.gpsimd).dma_start(
                            y.ap()[t * 128 : (t + 1) * 128, :], yt[:]
                        )
                return emit

            ytiles = {}
            for t in range(NT):
                ytiles[t] = y_pool.tile([128, C], f32, name=f"yt{t}", tag="yt")

            # ---------------- prologue production ---------------------------
            # First heads need qkT Mtiles 6 (both halves: keys) + 0 (query half 0),
            # and PV(h0) needs V tiles rolling in.
            u_qk(6, 0)(); u_qk(0, 0)(); u_qk(6, 1)()
            u_v(0, 0)(); u_v(1, 0)()

            # ---------------- filler schedules ------------------------------
            # NOTE: fillers are popped in emission order and the Tile framework
            # only enforces dependencies backwards in program order, so each
            # V half-tile unit must be EMITTED before the first PV matmul that
            # reads it (PV of head h at j-pair jp reads tiles 2jp,2jp+1 of the
            # heads hf*6..hf*6+6 half it belongs to).
            fill_qh0 = deque()
            for unit in (
                u_v(2, 0), u_derive(0), u_v(3, 0), u_v(4, 0), u_v(5, 0),
                u_v(6, 0), u_v(7, 0),
                lambda: w_mtile_res(7, nc.gpsimd, nc.gpsimd),
                u_derive(1),
                lambda: w_mtile_res(1, nc.gpsimd, nc.gpsimd),
                u_derive(2), u_derive(3),
                u_qk(7, 0),
                u_derive(4), u_derive(5),
                u_qk(1, 0),
                u_derive(6), u_derive(7),
                u_qk(7, 1),
                lambda: w_group(7, 1920, nc.gpsimd, nc.gpsimd),  # v heads 6-11
                u_v(0, 1), u_v(1, 1), u_v(2, 1), u_v(3, 1),
                lambda: w_mtile_res(8, nc.gpsimd, nc.gpsimd),
                lambda: w_mtile_res(2, nc.gpsimd, nc.gpsimd),
                u_qk(8, 0), u_qk(8, 1), u_qk(2, 0),
                u_v(4, 1), u_v(5, 1), u_v(6, 1), u_v(7, 1),
                lambda: w_mtile_res(9, nc.gpsimd, nc.gpsimd),
                lambda: w_mtile_res(3, nc.gpsimd, nc.gpsimd),
                u_qk(9, 0), u_qk(9, 1), u_qk(3, 0),
                lambda: w_mtile_res(10, nc.gpsimd, nc.gpsimd),
                lambda: w_mtile_res(4, nc.gpsimd, nc.gpsimd),
                u_qk(10, 0), u_qk(10, 1), u_qk(4, 0),
                lambda: w_mtile_res(11, nc.gpsimd, nc.gpsimd),
                lambda: w_mtile_res(5, nc.gpsimd, nc.gpsimd),
                u_qk(11, 0), u_qk(11, 1), u_qk(5, 0),
                u_qk(0, 1), u_qk(1, 1),
            ):
                fill_qh0.append(unit)
            fill_qh1 = deque()
            for unit in (
                u_qk(2, 1), u_qk(3, 1),
                u_wp(0), u_wp(1),
                u_qk(4, 1), u_qk(5, 1),
                u_brow(0), u_brow(1),
                u_proj(0, 0), u_proj(0, 1), u_proj(1, 0), u_proj(1, 1),
                u_proj(2, 0), u_proj(2, 1), u_proj(3, 0), u_proj(3, 1),
            ):
                fill_qh1.append(unit)

            # ---------------- attention spine -------------------------------
            for qh in range(2):
                fillers = fill_qh0 if qh == 0 else fill_qh1
                for h in range(H):
                    t, h2 = h // 2, h % 2
                    hsl = slice(h2 * 64, h2 * 64 + 64)
                    O = psO.tile([128, 512], f32, name=f"O{qh}_{h}", tag="O")
                    for jp in range(4):
                        S = psS.tile([128, 1024], f32, name=f"S{qh}_{h}_{jp}", tag="S")
                        for jj in range(2):
                            j = 2 * jp + jj
                            mm(S[:, jj * 512 : jj * 512 + 512],
                               qkTh[6 + t][j // 4][hsl, (j % 4) * 128 : (j % 4) * 128 + 128],
                               qkTh[t][qh][hsl, :],
                               start=True, stop=True)
                        if fillers:
                            fillers.popleft()()
                        P = p_pool.tile([128, 1024], bf16, name=f"P{qh}_{h}_{jp}", tag="P")
                        nc.scalar.activation(
                            P[:], S[:], Exp, bias=eshift_col[:, 0:1], scale=SCALE
                        )
                        for jj in range(2):
                            j = 2 * jp + jj
                            mm(O[:], vp[:, j, h, :], P[:, jj * 512 : jj * 512 + 512],
                               start=(jp == 0 and jj == 0),
                               stop=(jp == 3 and jj == 1))
                        if fillers:
                            fillers.popleft()()
                    inv = inv_pool.tile([64, 512], f32, name=f"inv{qh}_{h}", tag="inv")
                    nc.vector.reciprocal(inv[:], O[64:128, :])
                    nc.vector.tensor_tensor(
                        out=aot_v[qh][h2 * 64 : h2 * 64 + 64, t, :],
                        in0=O[0:64, :],
                        in1=inv[:],
                        op=mul_op,
                    )
                while fillers:
                    fillers.popleft()()

            # ---------------- tail: proj for query half 1 -------------------
            for t in range(4, NT):
                u_proj(t, 0)()
                u_proj(t, 1)()

    nc.compile()
    return nc


_NC_CACHE = {}


def _get_nc():
    nc = _NC_CACHE.get("nc")
    if nc is None:
        nc = build_nc()
        _NC_CACHE["nc"] = nc
    return nc


_RUNNER_CACHE = {}
_DEV_CACHE = {}


def _get_runner(n_cores=8):
    """Cached jitted 8-core executor (PJRT path, no per-call retrace)."""
    if n_cores in _RUNNER_CACHE:
        return _RUNNER_CACHE[n_cores]
    import jax
    from jax.sharding import Mesh, PartitionSpec
    from jax.experimental.shard_map import shard_map
    from concourse import mybir
    from concourse.bass2jax import (
        _bass_exec_p,
        install_neuronx_cc_hook,
        partition_id_tensor,
    )

    nc = _get_nc()
    install_neuronx_cc_hook()
    partition_name = nc.partition_id_tensor.name if nc.partition_id_tensor else None

    in_names, out_names, out_avals = [], [], []
    for alloc in nc.m.functions[0].allocations:
        if not isinstance(alloc, mybir.MemoryLocationSet):
            continue
        name = alloc.memorylocations[0].name
        if alloc.kind == "ExternalInput":
            if name != partition_name:
                in_names.append(name)
        elif alloc.kind == "ExternalOutput":
            out_names.append(name)
            out_avals.append(
                jax.core.ShapedArray(
                    tuple(alloc.tensor_shape), mybir.dt.np(alloc.dtype)
                )
            )
    all_in_names = list(in_names)
    if partition_name is not None:
        all_in_names.append(partition_name)

    def _body(*args):
        operands = list(args)
        if partition_name is not None:
            operands.append(partition_id_tensor())
        return tuple(
            _bass_exec_p.bind(
                *operands,
                out_avals=tuple(out_avals),
                in_names=tuple(all_in_names),
                out_names=tuple(out_names),
                lowering_input_output_aliases=(),
                sim_require_finite=False,
                sim_require_nnan=False,
                nc=nc,
            )
        )

    devices = jax.devices()[:n_cores]
    mesh = Mesh(np.asarray(devices), ("core",))
    in_specs = tuple(
        PartitionSpec("core") if n == "x" else PartitionSpec() for n in in_names
    )
    fn = jax.jit(
        shard_map(
            _body,
            mesh=mesh,
            in_specs=in_specs,
            out_specs=(PartitionSpec("core"),) * len(out_names),
            check_rep=False,
        ),
        keep_unused=True,
    )
    _RUNNER_CACHE[n_cores] = (fn, in_names, mesh)
    return _RUNNER_CACHE[n_cores]


def kernel(x, Wqkv, bqkv, Wproj, bproj):
    """Full-input entry point.

    x [8, 1024, 768] is sharded one batch element per NeuronCore (data
    parallel, weights replicated, no collectives); outputs are re-stacked.
    """
    x = np.ascontiguousarray(np.asarray(x, dtype=np.float32))
    Wqkv = np.ascontiguousarray(np.asarray(Wqkv, dtype=np.float32))
    bqkv = np.ascontiguousarray(np.asarray(bqkv, dtype=np.float32))
    Wproj = np.ascontiguousarray(np.asarray(Wproj, dtype=np.float32))
    bproj = np.ascontiguousarray(np.asarray(bproj, dtype=np.float32))
    B = x.shape[0]
    assert x.shape == (8, N, C), f"expected (8, {N}, {C}), got {x.shape}"

    arrays = {
        "x": x.reshape(B * N, C),
        "Wqkv": Wqkv,
        "bqkv": bqkv,
        "Wproj": Wproj,
        "bproj": bproj,
    }
    try:
        import jax
        from jax.sharding import NamedSharding, PartitionSpec

        fn, in_names, mesh = _get_runner(B)
        ops = []
        for n in in_names:
            a = arrays[n]
            if n == "x":
                ops.append(a)
                continue
            key = (n, id(a), a.shape)
            cached = _DEV_CACHE.get(n)
            if cached is None or cached[0] != key:
                dev = jax.device_put(a, NamedSharding(mesh, PartitionSpec()))
                _DEV_CACHE[n] = (key, dev, a)
                cached = _DEV_CACHE[n]
            ops.append(cached[1])
        outs = fn(*ops)
        yv = np.asarray(outs[0]).reshape(B, N, C)
        return yv.astype(np.float32)
    except Exception:
        from concourse import bass_utils

        nc = _get_nc()
        in_maps = [
            {
                "x": x[c],
                "Wqkv": Wqkv,
                "bqkv": bqkv,
                "Wproj": Wproj,
                "bproj": bproj,
            }
            for c in range(B)
        ]
        res = bass_utils.run_bass_kernel_spmd(nc, in_maps, core_ids=list(range(B)))
        return np.stack([res.results[c]["y"] for c in range(B)]).astype(np.float32)


# revision 50
# speedup vs baseline: 1.1468x; 1.0047x over previous
"""Multi-head self-attention Bass/Tile kernel for TRN2.

Per-core problem (batch sharded across 8 cores):
  x [N=1024, C=768], Wqkv [768, 2304], bqkv [2304], Wproj [768, 768], bproj [768]
  -> y [1024, 768]

Design (all matmuls bf16, f32 PSUM accumulation; attention restructured so the
ACT engine streams 96 exp tiles back-to-back while PE interleaves everything):

  x   --DMA-->  f32 tiles --DVE cast--> bf16 --DMA-xbar-transpose--> xt [128,6,1024]
  qkT [128,1024] bf16 per feature M-tile (12), produced in query-half chunks
  V   [token,128-col] bf16 per token tile, stored padded per head with a
      64-wide ones block (PV then yields softmax sums replicated on rows 64:128)
  S^T [key128, (2j) x 512q] per (head, query-half, j-pair) in PSUM
  P   = exp(S*0.125 - 2) bf16 (ACT; the -2 shift cancels in normalization and
      is there so an fp8 variant can't overflow; harmless here)
  O   [128, 512] PSUM accumulated over 8 key tiles; rows 64:128 = sums
  aot = O[0:64] * recip(O[64:128]) -> bf16 (DVE, per head)
  y   = aot.T @ Wproj + (bproj + bqkv_v @ Wproj)  (V-bias folded into proj bias)

Scheduling: the attention loop (2 query-halves x 12 heads x 4 j-pairs) is the
spine; qkT/V/proj production matmuls are drained from a filler deque between
S/PV groups so the PE stream stays dense while ACT exps run concurrently.
"""

import numpy as np
from collections import deque

N = 1024
C = 768
H = 12
D = 64
NT = 8    # token tiles
CT = 6    # channel tiles
SCALE = 1.0 / np.sqrt(D)
ESHIFT = -2.0


def build_nc():
    import concourse.bass as bass
    import concourse.tile as tile
    from concourse import bacc, mybir

    f32 = mybir.dt.float32
    bf16 = mybir.dt.bfloat16
    fp8 = mybir.dt.float8e4
    DR = mybir.MatmulPerfMode.DoubleRow
    Exp = mybir.ActivationFunctionType.Exp
    mul_op = mybir.AluOpType.mult
    add_op = mybir.AluOpType.add
    sub_op = mybir.AluOpType.subtract

    nc = bacc.Bacc(None, target_bir_lowering=False)

    x = nc.dram_tensor("x", [N, C], f32, kind="ExternalInput")
    wqkv = nc.dram_tensor("Wqkv", [C, 3 * C], f32, kind="ExternalInput")
    bqkv = nc.dram_tensor("bqkv", [3 * C], f32, kind="ExternalInput")
    wproj = nc.dram_tensor("Wproj", [C, C], f32, kind="ExternalInput")
    bproj = nc.dram_tensor("bproj", [C], f32, kind="ExternalInput")
    y = nc.dram_tensor("y", [N, C], f32, kind="ExternalOutput")

    mm = nc.tensor.matmul

    from contextlib import ExitStack
    with ExitStack() as _es:
        tc = _es.enter_context(tile.TileContext(nc))
        P = lambda **kw: _es.enter_context(tc.tile_pool(**kw))
        const = P(name="const", bufs=1)
        xst_pool = P(name="xst", bufs=3)
        xbf_pool = P(name="xbf", bufs=5)
        xt_pool = P(name="xt", bufs=1)
        x8_pool = P(name="x8", bufs=1)
        w64_pool = P(name="w64", bufs=2)
        wst_pool = P(name="wst", bufs=2)
        wq_pool = P(name="wq", bufs=6)
        wp_pool = P(name="wp", bufs=2)
        qk_pool = P(name="qk", bufs=24)
        vpad_pool = P(name="vpad", bufs=1)
        p_pool = P(name="pp", bufs=7)
        aot_pool = P(name="aot", bufs=1)
        inv_pool = P(name="inv", bufs=2)
        y_pool = P(name="yb", bufs=2)
        psS = P(name="psS", bufs=2, space="PSUM")
        psO = P(name="psO", bufs=2, space="PSUM")
        psP = P(name="psP", bufs=2, space="PSUM")
        if True:
            bq_cols = const.tile([128, 12], f32)
            nc.gpsimd.dma_start(
                bq_cols[:], bqkv.ap().rearrange("(m p) -> p m", p=128)[:, 0:12]
            )
            bv_col = const.tile([128, CT], f32)
            nc.gpsimd.dma_start(
                bv_col[:], bqkv.ap()[2 * C : 3 * C].rearrange("(k p) -> p k", p=128)
            )
            bv_bf = const.tile([128, CT], bf16)
            nc.vector.tensor_copy(bv_bf[:], bv_col[:])
            bp_row = const.tile([1, C], f32)
            brow_sb = const.tile([1, C], f32)
            brow_bf = const.tile([1, C], bf16)
            brow_bc = const.tile([128, C], bf16)
            eshift_col = const.tile([128, 1], f32)
            nc.vector.memset(eshift_col[:], ESHIFT)

            # vpad: [128 tok, (j 8) (h 12) (m 128)] bf16; cols 64:128 = ones
            vpad = vpad_pool.tile([128, NT * H * 128], bf16)
            vp = vpad[:].rearrange("p (j h m) -> p j h m", j=NT, h=H)
            nc.gpsimd.memset(vp[:, :, :, 64:128], 1.0)

            # ---------------- x pipeline: load, cast, xbar-transpose -------
            xt_all = xt_pool.tile([128, CT * N], bf16)
            xt_v = xt_all[:].rearrange("p (k n) -> p k n", k=CT)
            x0t = x8_pool.tile([128, CT * N], fp8, name="x0t", tag="x0t")
            x1t = x8_pool.tile([128, CT * N], fp8, name="x1t", tag="x1t")
            x0t_v = x0t[:].rearrange("p (k n) -> p k n", k=CT)
            x1t_v = x1t[:].rearrange("p (k n) -> p k n", k=CT)
            pending_T = []

            def u_derive(ti):  # xt -> fp8 residual pair for token tile ti
                def emit():
                    sl = slice(ti * 128, (ti + 1) * 128)
                    nc.vector.tensor_copy(x0t_v[:, :, sl], xt_v[:, :, sl])
                    nc.vector.tensor_tensor(
                        out=x1t_v[:, :, sl], in0=xt_v[:, :, sl],
                        in1=x0t_v[:, :, sl], op=sub_op,
                    )
                return emit

            def x_tile(ti, eng=None):
                xst = xst_pool.tile([128, C], f32, name=f"xst{ti}", tag="xst")
                (eng or nc.sync).dma_start(xst[:], x.ap()[ti * 128 : (ti + 1) * 128, :])
                xbf = xbf_pool.tile([128, C], bf16, name=f"xbf{ti}", tag="xbf")
                nc.vector.tensor_copy(xbf[:], xst[:])
                pending_T.append(
                    lambda ti=ti, xbf=xbf: nc.sync.dma_start_transpose(
                        xt_v[:, :, ti * 128 : (ti + 1) * 128], xbf[:]
                    )
                )

            # loads run ahead of transposes on the sync queue so a transpose's
            # wait (on the DVE cast) never delays a later x load; token half 0
            # (tiles 0-3) transposes first since S needs key-half 0 first
            for ti in range(4):
                x_tile(ti)
            while pending_T:
                pending_T.pop(0)()

            # ---------------- weight loads + casts -------------------------
            # Wqkv columns, bf16, split per column-group tile:
            #   groups 0..5: q|k M-tile pairs (256 cols each)
            #   groups 6,7:  v columns (384 each)
            wqg = {}   # v-column groups (6: cols 0:384, 7: cols 384:768)
            wqm = {}   # bf16 per-Mtile q|k weight tiles (prologue Mtiles 0, 6)
            wqr = {}   # residual fp8 pairs (w0, w1) of 64*W per Mtile, viewed
                       # [128, 3 cpairs, 2, 128] for DoubleRow lhsT

            def w_mtile(mi, eng_dma, eng_cast):
                st = wst_pool.tile([128, 2304], f32, name=f"wsm{mi}", tag="wst")
                stv = st[:, 0 : CT * 128].rearrange("p (k c) -> p k c", k=CT)
                eng_dma.dma_start(
                    stv,
                    wqkv.ap().rearrange("(k p) c -> p k c", p=128)[
                        :, :, mi * 128 : (mi + 1) * 128
                    ],
                )
                wt = wq_pool.tile(
                    [128, CT * 128], bf16, name=f"wm{mi}", tag="wm", bufs=2
                )
                eng_cast.tensor_copy(wt[:], st[:, 0 : CT * 128])
                wqm[mi] = wt[:].rearrange("p (k c) -> p k c", k=CT)

            def w_mtile_res(mi, eng_dma, eng_chain):
                # 64*W as an fp8 residual pair (the x64 dodges e4m3 subnormals
                # on W ~ N(0, C^-0.5); the qkT eviction scales psum by 1/64)
                st = wst_pool.tile([128, 2304], f32, name=f"wsm{mi}", tag="wst")
                stv = st[:, 0 : CT * 128].rearrange("p (k c) -> p k c", k=CT)
                eng_dma.dma_start(
                    stv,
                    wqkv.ap().rearrange("(k p) c -> p k c", p=128)[
                        :, :, mi * 128 : (mi + 1) * 128
                    ],
                )
                st64 = w64_pool.tile([128, CT * 128], f32, name=f"w64_{mi}", tag="w64")
                eng_chain.tensor_scalar_mul(st64[:], st[:, 0 : CT * 128], 64.0)
                w0 = wq_pool.tile([128, CT * 128], fp8, name=f"w0_{mi}", tag="w0", bufs=12)
                eng_chain.tensor_copy(w0[:], st64[:])
                w1 = wq_pool.tile([128, CT * 128], fp8, name=f"w1_{mi}", tag="w1", bufs=12)
                eng_chain.tensor_tensor(out=w1[:], in0=st64[:], in1=w0[:], op=sub_op)
                wqr[mi] = tuple(
                    w[:].rearrange("p (u i c) -> p u i c", u=3, i=2) for w in (w0, w1)
                )

            def w_group_dma(gi, c0, eng_dma, gate=None):
                st = wst_pool.tile([128, 2304], f32, name=f"wst{gi}", tag="wst")
                if gate is not None:
                    # dummy WAW write that depends on `gate` (an AP): delays
                    # this DMA's device-arrival until the gate data exists,
                    # keeping it from preempting earlier critical transfers
                    nc.gpsimd.tensor_copy(st[0:1, 0:1], gate)
                stv = st[:, 0 : CT * 384].rearrange("p (k c) -> p k c", k=CT)
                eng_dma.dma_start(
                    stv,
                    wqkv.ap().rearrange("(k p) c -> p k c", p=128)[:, :, c0 : c0 + 384],
                )
                return st

            def w_group_cast(gi, st, eng_cast):
                wt = wq_pool.tile(
                    [128, CT * 384], bf16, name=f"wq{gi}", tag="wq384", bufs=2
                )
                eng_cast.tensor_copy(wt[:], st[:, 0 : CT * 384])
                wqg[gi] = wt[:].rearrange("p (k c) -> p k c", k=CT)

            def w_group(gi, c0, eng_dma, eng_cast, gate=None):
                st = w_group_dma(gi, c0, eng_dma, gate)
                w_group_cast(gi, st, eng_cast)

            # First-needed M-tiles on the scalar queue right away; the rest of
            # the x pipeline; then the heads-0-5 V columns on sync after T7 so
            # the transposes (S key-half 1) aren't delayed.
            w_mtile(6, nc.scalar, nc.vector)
            w_mtile(0, nc.scalar, nc.vector)
            for ti in range(4, NT):
                x_tile(ti)
            while pending_T:
                pending_T.pop(0)()
            w_group(6, 1536, nc.sync, nc.vector)   # v cols 0:384 (heads 0-5)
            nc.scalar.dma_start(bp_row[:], bproj.ap()[None, :])

            def qk_lhsT(mi, k):  # [128, 128] W tile for feature M-tile mi
                return wqm[mi][:, k, :]

            # Wproj halves (deferred; loaded during attention)
            wpg = {}

            def wp_group(hi):
                st = wst_pool.tile([128, 2304], f32, name=f"wpst{hi}", tag="wst")
                stv = st[:, 0 : CT * 384].rearrange("p (k c) -> p k c", k=CT)
                nc.sync.dma_start(
                    stv,
                    wproj.ap().rearrange("(k p) c -> p k c", p=128)[
                        :, :, hi * 384 : (hi + 1) * 384
                    ],
                )
                wt = wp_pool.tile([128, CT * 384], bf16, name=f"wp{hi}", tag="wp")
                nc.gpsimd.tensor_copy(wt[:], st[:, 0 : CT * 384])
                wpg[hi] = wt[:].rearrange("p (k c) -> p k c", k=CT)

            # ---------------- persistent bf16 activation tiles -------------
            # qkT half-tiles: qkTh[mi][half] = [128, 512] (tokens half*512..)
            qkTh = [
                [
                    qk_pool.tile([128, 512], bf16, name=f"qkT{mi}_{hf}", tag="qkT")
                    for hf in range(2)
                ]
                for mi in range(12)
            ]
            aot = [aot_pool.tile([128, CT * 512], bf16, name=f"aot{qh}") for qh in range(2)]
            aot_v = [a[:].rearrange("p (k n) -> p k n", k=CT) for a in aot]

            # ---------------- production units (filler closures) -----------
            def u_qk(mi, hf):
                def emit():
                    ps = psP.tile([128, 512], f32, name=f"qps{mi}_{hf}", tag="ps")
                    sl = slice(hf * 512, hf * 512 + 512)
                    if mi in wqm:  # bf16 path (prologue M-tiles)
                        for k in range(CT):
                            mm(ps[:], qk_lhsT(mi, k), xt_v[:, k, sl],
                               start=(k == 0), stop=(k == CT - 1))
                        nc.vector.tensor_scalar_add(
                            qkTh[mi][hf][:], ps[:], bq_cols[:, mi : mi + 1]
                        )
                        return
                    # fp8 residual DoubleRow path: 64*(q|k) = x0W0 + x1W0 + x0W1
                    w0, w1 = wqr[mi]
                    passes = ((w0, x0t_v), (w0, x1t_v), (w1, x0t_v))
                    n9 = 0
                    for wv, xv in passes:
                        for u in range(3):
                            mm(ps[:], wv[:, u, :, :],
                               xv[:, 2 * u : 2 * u + 2, sl],
                               start=(n9 == 0), stop=(n9 == 8), perf_mode=DR)
                            n9 += 1
                    nc.vector.tensor_scalar(
                        qkTh[mi][hf][:], ps[:], 1.0 / 64.0,
                        bq_cols[:, mi : mi + 1], mul_op, add_op,
                    )
                return emit

            def u_v(ti, hf):  # V chunk: channels hf*384..+384 = heads hf*6..+6
                def emit():
                    ps = psP.tile([128, 512], f32, name=f"vps{ti}_{hf}", tag="ps")
                    for k in range(CT):
                        mm(ps[:, 0:384], xt_v[:, k, ti * 128 : (ti + 1) * 128],
                           wqg[6 + hf][:, k, :],
                           start=(k == 0), stop=(k == CT - 1))
                    nc.vector.tensor_copy(
                        vp[:, ti, hf * 6 : hf * 6 + 6, 0:64],
                        ps[:, 0:384].rearrange("p (h d) -> p h d", h=6),
                    )
                return emit

            def u_wp(hi):
                return lambda: wp_group(hi)

            def u_brow(hi):
                def emit():
                    ps = psP.tile([128, 512], f32, name=f"brps{hi}", tag="ps")
                    for k in range(CT):
                        mm(ps[0:1, 0:384], bv_bf[:, k : k + 1], wpg[hi][:, k, :],
                           start=(k == 0), stop=(k == CT - 1))
                    nc.vector.tensor_tensor(
                        out=brow_sb[:, hi * 384 : (hi + 1) * 384],
                        in0=ps[0:1, 0:384],
                        in1=bp_row[:, hi * 384 : (hi + 1) * 384],
                        op=add_op,
                    )
                    if hi == 1:
                        nc.vector.tensor_copy(brow_bf[:], brow_sb[:])
                        nc.gpsimd.partition_broadcast(brow_bc[:], brow_bf[:])
                return emit

            def u_proj(t, hi):  # token tile t, output cols hi*384..+384
                def emit():
                    qh, tt = t // 4, t % 4
                    pool = psO if t >= 4 else psP
                    ps = pool.tile([128, 512], f32, name=f"yps{t}_{hi}",
                                   tag=("O" if t >= 4 else "ps"))
                    for k in range(CT):
                        mm(ps[:, 0:384], aot_v[qh][:, k, tt * 128 : (tt + 1) * 128],
                           wpg[hi][:, k, :],
                           start=(k == 0), stop=(k == CT - 1))
                    yt = ytiles[t]
                    nc.vector.tensor_tensor(
                        out=yt[:, hi * 384 : (hi + 1) * 384],
                        in0=ps[:, 0:384],
                        in1=brow_bc[:, hi * 384 : (hi + 1) * 384],
                        op=add_op,
                    )
                    (nc.sync if t % 2 == 0 else nc.gpsimd).dma_start(
                        y.ap()[t * 128 : (t + 1) * 128, hi * 384 : (hi + 1) * 384],
                        yt[:, hi * 384 : (hi + 1) * 384],
                    )
                return emit

            ytiles = {}
            for t in range(NT):
                ytiles[t] = y_pool.tile([128, C], f32, name=f"yt{t}", tag="yt")

            # ---------------- prologue production ---------------------------
            # First heads need qkT Mtiles 6 (both halves: keys) + 0 (query half 0),
            # and PV(h0) needs V tiles rolling in.
            u_qk(6, 0)(); u_qk(0, 0)(); u_qk(6, 1)()

            # ---------------- filler schedules ------------------------------
            # NOTE: fillers are popped in emission order and the Tile framework
            # only enforces dependencies backwards in program order, so each
            # V half-tile unit must be EMITTED before the first PV matmul that
            # reads it (PV of head h at j-pair jp reads tiles 2jp,2jp+1 of the
            # heads hf*6..hf*6+6 half it belongs to).
            fill_qh0 = deque()
            for unit in (
                u_derive(0), u_derive(1), u_derive(2), u_derive(3),
                lambda: w_mtile_res(7, nc.gpsimd, nc.gpsimd),
                lambda: w_mtile_res(1, nc.gpsimd, nc.gpsimd),
                u_derive(4), u_derive(5), u_derive(6), u_derive(7),
                u_qk(7, 0), u_qk(1, 0), u_qk(7, 1),
                lambda: w_mtile_res(8, nc.gpsimd, nc.gpsimd),
                lambda: w_mtile_res(2, nc.gpsimd, nc.gpsimd),
                lambda: w_group(7, 1920, nc.gpsimd, nc.gpsimd),  # v heads 6-11
                u_v(0, 0), u_v(1, 0), u_v(2, 0), u_v(3, 0), u_v(4, 0),
                u_v(5, 0), u_v(6, 0), u_v(7, 0),
                u_v(0, 1), u_v(1, 1), u_v(2, 1), u_v(3, 1),
                u_qk(8, 0), u_qk(8, 1), u_qk(2, 0),
                u_v(4, 1), u_v(5, 1), u_v(6, 1), u_v(7, 1),
                lambda: w_mtile_res(9, nc.gpsimd, nc.gpsimd),
                lambda: w_mtile_res(3, nc.gpsimd, nc.gpsimd),
                u_qk(9, 0), u_qk(9, 1), u_qk(3, 0),
                lambda: w_mtile_res(10, nc.gpsimd, nc.gpsimd),
                lambda: w_mtile_res(4, nc.gpsimd, nc.gpsimd),
                u_qk(10, 0), u_qk(10, 1), u_qk(4, 0),
                lambda: w_mtile_res(11, nc.gpsimd, nc.gpsimd),
                lambda: w_mtile_res(5, nc.gpsimd, nc.gpsimd),
                u_qk(11, 0), u_qk(11, 1), u_qk(5, 0),
                u_qk(0, 1), u_qk(1, 1),
            ):
                fill_qh0.append(unit)
            fill_qh1 = deque()
            for unit in (
                u_qk(2, 1), u_qk(3, 1),
                u_wp(0), u_wp(1),
                u_qk(4, 1), u_qk(5, 1),
                u_brow(0), u_brow(1),
                u_proj(0, 0), u_proj(0, 1), u_proj(1, 0), u_proj(1, 1),
                u_proj(2, 0), u_proj(2, 1), u_proj(3, 0), u_proj(3, 1),
            ):
                fill_qh1.append(unit)

            # ---------------- attention spine -------------------------------
            def emit_S(qh, h, jp):
                t, h2 = h // 2, h % 2
                hsl = slice(h2 * 64, h2 * 64 + 64)
                S = psS.tile([128, 1024], f32, name=f"S{qh}_{h}_{jp}", tag="S")
                for jj in range(2):
                    j = 2 * jp + jj
                    mm(S[:, jj * 512 : jj * 512 + 512],
                       qkTh[6 + t][j // 4][hsl, (j % 4) * 128 : (j % 4) * 128 + 128],
                       qkTh[t][qh][hsl, :],
                       start=True, stop=True)
                return S

            def emit_exp(qh, h, jp, S):
                P = p_pool.tile([128, 1024], bf16, name=f"P{qh}_{h}_{jp}", tag="P")
                nc.scalar.activation(
                    P[:], S[:], Exp, bias=eshift_col[:, 0:1], scale=SCALE
                )
                return P

            def emit_PV(h, jp, P, O):
                for jj in range(2):
                    mm(O[:], vp[:, 2 * jp + jj, h, :], P[:, jj * 512 : jj * 512 + 512],
                       start=(jp == 0 and jj == 0), stop=(jp == 3 and jj == 1))

            def o_norm(qh, h, O):
                t, h2 = h // 2, h % 2
                inv = inv_pool.tile([64, 512], f32, name=f"inv{qh}_{h}", tag="inv")
                nc.vector.reciprocal(inv[:], O[64:128, :])
                nc.vector.tensor_tensor(
                    out=aot_v[qh][h2 * 64 : h2 * 64 + 64, t, :],
                    in0=O[0:64, :],
                    in1=inv[:],
                    op=mul_op,
                )

            for qh in range(2):
                fillers = fill_qh0 if qh == 0 else fill_qh1

                def pop(n=1):
                    for _ in range(n):
                        if fillers:
                            fillers.popleft()()

                h0 = 0
                if qh == 0:
                    # Opening: stream S+exp for heads 0-1 before any PV so the
                    # PE never head-of-line blocks on the V pipeline (V weights
                    # land ~4us after the first exp).
                    Ot, Pt = {}, {}
                    for hh in (0, 1):
                        Ot[hh] = psO.tile([128, 512], f32, name=f"O0_{hh}", tag="O")
                        for jp in range(4):
                            S = emit_S(0, hh, jp)
                            pop(1)
                            Pt[(hh, jp)] = emit_exp(0, hh, jp, S)
                            pop(1)
                    pop(2)
                    for hh in (0, 1):
                        for jp in range(4):
                            pop(2 if hh == 0 else 1)
                            emit_PV(hh, jp, Pt[(hh, jp)], Ot[hh])
                        o_norm(0, hh, Ot[hh])
                    h0 = 2
                for h in range(h0, H):
                    O = psO.tile([128, 512], f32, name=f"O{qh}_{h}", tag="O")
                    for jp in range(4):
                        S = emit_S(qh, h, jp)
                        pop(1)
                        P = emit_exp(qh, h, jp, S)
                        emit_PV(h, jp, P, O)
                        pop(1)
                    o_norm(qh, h, O)
                pop(len(fillers))

            # ---------------- tail: proj for query half 1 -------------------
            for t in range(4, NT):
                u_proj(t, 0)()
                u_proj(t, 1)()

    nc.compile()
    return nc


_NC_CACHE = {}


def _get_nc():
    nc = _NC_CACHE.get("nc")
    if nc is None:
        nc = build_nc()
        _NC_CACHE["nc"] = nc
    return nc


_RUNNER_CACHE = {}
_DEV_CACHE = {}


def _get_runner(n_cores=8):
    """Cached jitted 8-core executor (PJRT path, no per-call retrace)."""
    if n_cores in _RUNNER_CACHE:
        return _RUNNER_CACHE[n_cores]
    import jax
    from jax.sharding import Mesh, PartitionSpec
    from jax.experimental.shard_map import shard_map
    from concourse import mybir
    from concourse.bass2jax import (
        _bass_exec_p,
        install_neuronx_cc_hook,
        partition_id_tensor,
    )

    nc = _get_nc()
    install_neuronx_cc_hook()
    partition_name = nc.partition_id_tensor.name if nc.partition_id_tensor else None

    in_names, out_names, out_avals = [], [], []
    for alloc in nc.m.functions[0].allocations:
        if not isinstance(alloc, mybir.MemoryLocationSet):
            continue
        name = alloc.memorylocations[0].name
        if alloc.kind == "ExternalInput":
            if name != partition_name:
                in_names.append(name)
        elif alloc.kind == "ExternalOutput":
            out_names.append(name)
            out_avals.append(
                jax.core.ShapedArray(
                    tuple(alloc.tensor_shape), mybir.dt.np(alloc.dtype)
                )
            )
    all_in_names = list(in_names)
    if partition_name is not None:
        all_in_names.append(partition_name)

    def _body(*args):
        operands = list(args)
        if partition_name is not None:
            operands.append(partition_id_tensor())
        return tuple(
            _bass_exec_p.bind(
                *operands,
                out_avals=tuple(out_avals),
                in_names=tuple(all_in_names),
                out_names=tuple(out_names),
                lowering_input_output_aliases=(),
                sim_require_finite=False,
                sim_require_nnan=False,
                nc=nc,
            )
        )

    devices = jax.devices()[:n_cores]
    mesh = Mesh(np.asarray(devices), ("core",))
    in_specs = tuple(
        PartitionSpec("core") if n == "x" else PartitionSpec() for n in in_names
    )
    fn = jax.jit(
        shard_map(
            _body,
            mesh=mesh,
            in_specs=in_specs,
            out_specs=(PartitionSpec("core"),) * len(out_names),
            check_rep=False,
        ),
        keep_unused=True,
    )
    _RUNNER_CACHE[n_cores] = (fn, in_names, mesh)
    return _RUNNER_CACHE[n_cores]


def kernel(x, Wqkv, bqkv, Wproj, bproj):
    """Full-input entry point.

    x [8, 1024, 768] is sharded one batch element per NeuronCore (data
    parallel, weights replicated, no collectives); outputs are re-stacked.
    """
    x = np.ascontiguousarray(np.asarray(x, dtype=np.float32))
    Wqkv = np.ascontiguousarray(np.asarray(Wqkv, dtype=np.float32))
    bqkv = np.ascontiguousarray(np.asarray(bqkv, dtype=np.float32))
    Wproj = np.ascontiguousarray(np.asarray(Wproj, dtype=np.float32))
    bproj = np.ascontiguousarray(np.asarray(bproj, dtype=np.float32))
    B = x.shape[0]
    assert x.shape == (8, N, C), f"expected (8, {N}, {C}), got {x.shape}"

    arrays = {
        "x": x.reshape(B * N, C),
        "Wqkv": Wqkv,
        "bqkv": bqkv,
        "Wproj": Wproj,
        "bproj": bproj,
    }
    try:
        import jax
        from jax.sharding import NamedSharding, PartitionSpec

        fn, in_names, mesh = _get_runner(B)
        ops = []
        for n in in_names:
            a = arrays[n]
            if n == "x":
                ops.append(a)
                continue
            key = (n, id(a), a.shape)
            cached = _DEV_CACHE.get(n)
            if cached is None or cached[0] != key:
                dev = jax.device_put(a, NamedSharding(mesh, PartitionSpec()))
                _DEV_CACHE[n] = (key, dev, a)
                cached = _DEV_CACHE[n]
            ops.append(cached[1])
        outs = fn(*ops)
        yv = np.asarray(outs[0]).reshape(B, N, C)
        return yv.astype(np.float32)
    except Exception:
        from concourse import bass_utils

        nc = _get_nc()
        in_maps = [
            {
                "x": x[c],
                "Wqkv": Wqkv,
                "bqkv": bqkv,
                "Wproj": Wproj,
                "bproj": bproj,
            }
            for c in range(B)
        ]
        res = bass_utils.run_bass_kernel_spmd(nc, in_maps, core_ids=list(range(B)))
        return np.stack([res.results[c]["y"] for c in range(B)]).astype(np.float32)
